# revision 5
# baseline (speedup 1.0000x reference)
"""Trainium2 Bass kernel for nn_Agentembedding (cross-attention agent embedding).

Reference computation (per batch b):
    q = f_c @ Wq + bq                  # [256, 512]
    k = f @ Wk + bk                    # [4096, 512]
    v = f @ Wv + bv                    # [4096, 512]
    u = (k @ q^T) / sqrt(512)          # [4096, 256]
    p = softmax(u, axis=0)             # over the 4096 nodes
    out = p^T @ v                      # [256, 512]

Optimizations used here:
  * Data parallel over batch: 32 batches -> 4 per NeuronCore across 8 cores.
  * Low-rank associativity: since Q=256 < 512,
        u = f @ G  with G = Wk @ (s*q)^T           (never materialize k)
        num = (p^T @ f) @ Wv                       (never materialize v)
    cutting matmul FLOPs ~5x vs the naive order.
  * Softmax-invariance: per-query constants cancel, so the bk.q logit term
    and the max-subtraction are dropped (logits have tiny magnitude), and
    bv is added after the normalization: out = zT^T@Wv / S + bv.
  * bf16 matmul inputs (fp32 PSUM accumulation), activations pre-cast and
    pre-transposed on host so no on-chip transposes are needed.
"""

import sys

sys.path.insert(0, "/opt/trn_rl_repo")

import math
from contextlib import ExitStack

import ml_dtypes
import numpy as np

import concourse.bass as bass
import concourse.tile as tile
from concourse import bacc, mybir
from concourse.bass_utils import run_bass_kernel_spmd

BF16 = ml_dtypes.bfloat16

B, Q, N, D, K, V = 32, 256, 4096, 512, 512, 512
D2 = 2 * D  # f_c feature dim (1024)
NCORES = 8
BPC = B // NCORES  # batches per core
NT = 512  # node tile (outer); 4 sub-tiles of 128 inside

f32 = mybir.dt.float32
bf16 = mybir.dt.bfloat16
AF = mybir.ActivationFunctionType


def _emit(nc, tc, ctx, fcT_d, fT_d, fn_d, wq_d, wkT_d, wv_d, bq_d, bvb_d, out_d):
    const = ctx.enter_context(tc.tile_pool(name="const", bufs=1))
    fcT_p = ctx.enter_context(tc.tile_pool(name="fcT", bufs=2))
    qTsb_p = ctx.enter_context(tc.tile_pool(name="qTsb", bufs=2))
    Gsb_p = ctx.enter_context(tc.tile_pool(name="Gsb", bufs=2))
    fT_p = ctx.enter_context(tc.tile_pool(name="fTp", bufs=3))
    fn_p = ctx.enter_context(tc.tile_pool(name="fnp", bufs=3))
    p_p = ctx.enter_context(tc.tile_pool(name="pp", bufs=4))
    ztsb_p = ctx.enter_context(tc.tile_pool(name="ztsb", bufs=2))
    osb_p = ctx.enter_context(tc.tile_pool(name="osb", bufs=2))
    small_p = ctx.enter_context(tc.tile_pool(name="small", bufs=2))
    # PSUM budget (8 banks of [128, 2KB]):
    #   shared tag {qT, G, out}: 2 banks; zt: 2; s: 1; u double-buffered: 2.
    ps_share = ctx.enter_context(tc.tile_pool(name="ps_share", bufs=1, space="PSUM"))
    ps_zt = ctx.enter_context(tc.tile_pool(name="ps_zt", bufs=1, space="PSUM"))
    ps_s = ctx.enter_context(tc.tile_pool(name="ps_s", bufs=1, space="PSUM"))
    ps_u = ctx.enter_context(tc.tile_pool(name="ps_u", bufs=2, space="PSUM"))

    # ---- constants ----
    wq_sb = const.tile([128, 8 * K], bf16)  # chunk c of d2 -> cols [c*K, (c+1)*K)
    wkT_sb = const.tile([128, 4 * D], bf16)  # chunk c of key -> cols [c*D, (c+1)*D)
    wv_sb = const.tile([128, 4 * V], bf16)  # chunk c of d  -> cols [c*V, (c+1)*V)
    bq_sb = const.tile([128, 4], f32)
    bvb_sb = const.tile([128, V], f32)
    ones_sb = const.tile([128, 1], bf16)
    nc.vector.memset(ones_sb[:], 1.0)
    for c in range(8):
        nc.sync.dma_start(wq_sb[:, c * K:(c + 1) * K], wq_d[c * 128:(c + 1) * 128, :])
    for c in range(4):
        nc.sync.dma_start(wkT_sb[:, c * D:(c + 1) * D], wkT_d[c * 128:(c + 1) * 128, :])
        nc.sync.dma_start(wv_sb[:, c * V:(c + 1) * V], wv_d[c * 128:(c + 1) * 128, :])
    nc.sync.dma_start(bq_sb[:], bq_d[:])
    nc.sync.dma_start(bvb_sb[:], bvb_d[:])

    for b in range(BPC):
        # ---- load f_c^T (scaled q weights folded on host) ----
        fcT_sb = fcT_p.tile([128, 8 * Q], bf16)
        for c in range(8):
            nc.sync.dma_start(
                fcT_sb[:, c * Q:(c + 1) * Q], fcT_d[b, c * 128:(c + 1) * 128, :]
            )

        # ---- phase A: qT[key, q'] = (s*Wq)^T @ f_c^T + s*bq ----
        qT_ps = ps_share.tile([128, 4 * Q], f32, tag="share")
        for m in range(4):
            for c in range(8):
                nc.tensor.matmul(
                    qT_ps[:, m * Q:(m + 1) * Q],
                    wq_sb[:, c * K + m * 128: c * K + (m + 1) * 128],
                    fcT_sb[:, c * Q:(c + 1) * Q],
                    start=(c == 0),
                    stop=(c == 7),
                )
        qT_sb = qTsb_p.tile([128, 4 * Q], bf16)
        for m in range(4):
            nc.scalar.activation(
                qT_sb[:, m * Q:(m + 1) * Q],
                qT_ps[:, m * Q:(m + 1) * Q],
                AF.Identity,
                bias=bq_sb[:, m:m + 1],
            )

        # ---- phase B: G[d, q'] = Wk @ qT  (lhsT = Wk^T) ----
        G_ps = ps_share.tile([128, 4 * Q], f32, tag="share")
        for dt_ in range(4):
            for c in range(4):
                nc.tensor.matmul(
                    G_ps[:, dt_ * Q:(dt_ + 1) * Q],
                    wkT_sb[:, c * D + dt_ * 128: c * D + (dt_ + 1) * 128],
                    qT_sb[:, c * Q:(c + 1) * Q],
                    start=(c == 0),
                    stop=(c == 3),
                )
        G_sb = Gsb_p.tile([128, 4 * Q], bf16)
        for h in range(2):
            nc.scalar.copy(G_sb[:, h * 512:(h + 1) * 512], G_ps[:, h * 512:(h + 1) * 512])

        # ---- phase C: stream node tiles ----
        zt_ps = ps_zt.tile([128, 4 * Q], f32)  # zT[d, q'] accumulator
        s_ps = ps_s.tile([128, 2], f32)  # S[q'] accumulator (2 q'-tiles)
        for t in range(N // NT):
            fT_t = fT_p.tile([128, 4 * NT], bf16)
            for c in range(4):
                nc.sync.dma_start(
                    fT_t[:, c * NT:(c + 1) * NT],
                    fT_d[b, c * 128:(c + 1) * 128, t * NT:(t + 1) * NT],
                )
            fn_t = fn_p.tile([128, 4 * D], bf16)
            for s_ in range(4):
                nc.sync.dma_start(
                    fn_t[:, s_ * D:(s_ + 1) * D],
                    fn_d[b, t * NT + s_ * 128: t * NT + (s_ + 1) * 128, :],
                )
            for s_ in range(4):
                first = t == 0 and s_ == 0
                last = t == (N // NT - 1) and s_ == 3
                u_ps = ps_u.tile([128, Q], f32)
                for c in range(4):
                    nc.tensor.matmul(
                        u_ps[:],
                        fT_t[:, c * NT + s_ * 128: c * NT + (s_ + 1) * 128],
                        G_sb[:, c * Q:(c + 1) * Q],
                        start=(c == 0),
                        stop=(c == 3),
                    )
                p_sb = p_p.tile([128, Q], bf16)
                nc.scalar.activation(p_sb[:], u_ps[:], AF.Exp)
                # zt quarters share PSUM banks in pairs (256 f32 cols = half
                # a bank): a start=True pending-zeroes the whole 2KB bank, so
                # only the first quarter in each bank starts the group and
                # only the last one stops it.
                for dt_ in range(4):
                    nc.tensor.matmul(
                        zt_ps[:, dt_ * Q:(dt_ + 1) * Q],
                        fn_t[:, s_ * D + dt_ * 128: s_ * D + (dt_ + 1) * 128],
                        p_sb[:],
                        start=first and dt_ % 2 == 0,
                        stop=last and dt_ % 2 == 1,
                    )
                # both S columns live in one bank: start on qt0, stop on qt1
                for qt in range(2):
                    nc.tensor.matmul(
                        s_ps[:, qt:qt + 1],
                        p_sb[:, qt * 128:(qt + 1) * 128],
                        ones_sb[:],
                        start=first and qt == 0,
                        stop=last and qt == 1,
                    )

        # ---- phase D: out = zT^T @ Wv / S + bv ----
        zT_sb = ztsb_p.tile([128, 4 * Q], bf16)
        for h in range(2):
            nc.scalar.copy(zT_sb[:, h * 512:(h + 1) * 512], zt_ps[:, h * 512:(h + 1) * 512])
        s_sb = small_p.tile([128, 2], f32, tag="ssb")
        nc.vector.tensor_copy(s_sb[:], s_ps[:])
        r_sb = small_p.tile([128, 2], f32, tag="rsb")
        nc.vector.reciprocal(r_sb[:], s_sb[:])
        out_ps = ps_share.tile([128, 2 * V], f32, tag="share")
        for qt in range(2):
            for c in range(4):
                nc.tensor.matmul(
                    out_ps[:, qt * V:(qt + 1) * V],
                    zT_sb[:, c * Q + qt * 128: c * Q + (qt + 1) * 128],
                    wv_sb[:, c * V:(c + 1) * V],
                    start=(c == 0),
                    stop=(c == 3),
                )
        for qt in range(2):
            o_sb = osb_p.tile([128, V], f32)
            nc.vector.tensor_scalar_mul(o_sb[:], out_ps[:, qt * V:(qt + 1) * V], r_sb[:, qt:qt + 1])
            nc.vector.tensor_add(o_sb[:], o_sb[:], bvb_sb[:])
            nc.sync.dma_start(out_d[b, qt * 128:(qt + 1) * 128, :], o_sb[:])


_NC_CACHE = None


def build_nc():
    global _NC_CACHE
    if _NC_CACHE is not None:
        return _NC_CACHE
    nc = bacc.Bacc("TRN2", target_bir_lowering=False, debug=False)
    fcT_d = nc.declare_dram_parameter("fcT", [BPC, D2, Q], bf16, isOutput=False)
    fT_d = nc.declare_dram_parameter("fT", [BPC, D, N], bf16, isOutput=False)
    fn_d = nc.declare_dram_parameter("fn", [BPC, N, D], bf16, isOutput=False)
    wq_d = nc.declare_dram_parameter("wq", [D2, K], bf16, isOutput=False)
    wkT_d = nc.declare_dram_parameter("wkT", [K, D], bf16, isOutput=False)
    wv_d = nc.declare_dram_parameter("wv", [D, V], bf16, isOutput=False)
    bq_d = nc.declare_dram_parameter("bq", [128, 4], f32, isOutput=False)
    bvb_d = nc.declare_dram_parameter("bvb", [128, V], f32, isOutput=False)
    out_d = nc.declare_dram_parameter("out", [BPC, Q, V], f32, isOutput=True)
    with tile.TileContext(nc) as tc:
        with ExitStack() as ctx:
            _emit(nc, tc, ctx, fcT_d, fT_d, fn_d, wq_d, wkT_d, wv_d, bq_d, bvb_d, out_d)
    nc.compile()
    _NC_CACHE = nc
    return nc


def make_in_maps(f_c, f, Wq, bq, Wk, bk, Wv, bv):
    s = 1.0 / math.sqrt(K)
    f_c = np.asarray(f_c, dtype=np.float32)
    f = np.asarray(f, dtype=np.float32)
    wq_h = (np.asarray(Wq, dtype=np.float32) * s).astype(BF16)
    wkT_h = np.ascontiguousarray(np.asarray(Wk, dtype=np.float32).T).astype(BF16)
    wv_h = np.asarray(Wv, dtype=np.float32).astype(BF16)
    bq_h = np.ascontiguousarray(
        (np.asarray(bq, dtype=np.float32) * s).reshape(4, 128).T
    ).astype(np.float32)
    bvb_h = np.ascontiguousarray(
        np.broadcast_to(np.asarray(bv, dtype=np.float32), (128, V))
    )
    fn_bf = f.astype(BF16)  # [B, N, D]
    fT_bf = np.ascontiguousarray(fn_bf.transpose(0, 2, 1))  # [B, D, N]
    fcT_bf = np.ascontiguousarray(f_c.astype(BF16).transpose(0, 2, 1))  # [B, D2, Q]
    in_maps = []
    for core in range(NCORES):
        sl = slice(core * BPC, (core + 1) * BPC)
        in_maps.append(
            {
                "fcT": np.ascontiguousarray(fcT_bf[sl]),
                "fT": np.ascontiguousarray(fT_bf[sl]),
                "fn": np.ascontiguousarray(fn_bf[sl]),
                "wq": wq_h,
                "wkT": wkT_h,
                "wv": wv_h,
                "bq": bq_h,
                "bvb": bvb_h,
            }
        )
    return in_maps


def run(f_c, f, Wq, bq, Wk, bk, Wv, bv, **spmd_kwargs):
    nc = build_nc()
    in_maps = make_in_maps(f_c, f, Wq, bq, Wk, bk, Wv, bv)
    res = run_bass_kernel_spmd(nc, in_maps, list(range(NCORES)), **spmd_kwargs)
    out = np.concatenate([res.results[c]["out"] for c in range(NCORES)], axis=0)
    return out.astype(np.float32), res


def kernel(f_c, f, Wq, bq, Wk, bk, Wv, bv):
    out, _ = run(f_c, f, Wq, bq, Wk, bk, Wv, bv)
    return out


# revision 7
# speedup vs baseline: 1.1490x; 1.1490x over previous
"""Trainium2 Bass kernel for nn_Agentembedding (cross-attention agent embedding).

Reference computation (per batch b):
    q = f_c @ Wq + bq                  # [256, 512]
    k = f @ Wk + bk                    # [4096, 512]
    v = f @ Wv + bv                    # [4096, 512]
    u = (k @ q^T) / sqrt(512)          # [4096, 256]
    p = softmax(u, axis=0)             # over the 4096 nodes
    out = p^T @ v                      # [256, 512]

Optimizations used here:
  * Data parallel over batch: 32 batches -> 4 per NeuronCore across 8 cores.
  * Low-rank associativity: since Q=256 < 512,
        u = f @ G  with G = Wk @ (s*q)^T           (never materialize k)
        num = (p^T @ f) @ Wv                       (never materialize v)
    cutting matmul FLOPs ~5x vs the naive order.
  * Softmax-invariance: per-query constants cancel, so the bk.q logit term
    and the max-subtraction are dropped (logits have tiny magnitude), and
    bv is added after the normalization: out = zT^T@Wv / S + bv.
  * bf16 matmul inputs (fp32 PSUM accumulation), activations pre-cast and
    pre-transposed on host so no on-chip transposes are needed.
  * S (softmax denominators) accumulated on DVE (p-tile adds) with a final
    128-lane fold matmul, instead of per-tile N=1 matmuls on PE.
  * u(i+1) matmuls emitted before zt(i) so PE covers the exp(i) latency.
"""

import sys

sys.path.insert(0, "/opt/trn_rl_repo")

import math
from contextlib import ExitStack

import ml_dtypes
import numpy as np

import concourse.bass as bass
import concourse.tile as tile
from concourse import bacc, mybir
from concourse.bass_utils import run_bass_kernel_spmd

BF16 = ml_dtypes.bfloat16

B, Q, N, D, K, V = 32, 256, 4096, 512, 512, 512
D2 = 2 * D  # f_c feature dim (1024)
NCORES = 8
BPC = B // NCORES  # batches per core
NT = 512  # node tile (outer); 4 sub-tiles of 128 inside
NSUB = N // 128  # 32 sub-tiles per batch

f32 = mybir.dt.float32
bf16 = mybir.dt.bfloat16
AF = mybir.ActivationFunctionType


class _Emitter:
    def __init__(self, nc, tc, ctx, tensors):
        self.nc = nc
        self.tc = tc
        (self.fcT_d, self.fT_d, self.fn_d, self.wq_d, self.wkT_d, self.wv_d,
         self.bq_d, self.bvb_d, self.out_d) = tensors

        self.const = ctx.enter_context(tc.tile_pool(name="const", bufs=1))
        self.fcT_p = ctx.enter_context(tc.tile_pool(name="fcT", bufs=2))
        self.qTsb_p = ctx.enter_context(tc.tile_pool(name="qTsb", bufs=2))
        self.Gsb_p = ctx.enter_context(tc.tile_pool(name="Gsb", bufs=2))
        self.fT_p = ctx.enter_context(tc.tile_pool(name="fTp", bufs=3))
        self.fn_p = ctx.enter_context(tc.tile_pool(name="fnp", bufs=3))
        self.p_p = ctx.enter_context(tc.tile_pool(name="pp", bufs=4))
        self.sacc_p = ctx.enter_context(tc.tile_pool(name="sacc", bufs=2))
        self.ztsb_p = ctx.enter_context(tc.tile_pool(name="ztsb", bufs=2))
        self.osb_p = ctx.enter_context(tc.tile_pool(name="osb", bufs=2))
        self.small_p = ctx.enter_context(tc.tile_pool(name="small", bufs=2))
        # PSUM budget (8 banks):
        #   qT (half at a time): 1; {G, out} shared tag: 2; zt: 2; u: 3.
        self.ps_qt = ctx.enter_context(tc.tile_pool(name="ps_qt", bufs=1, space="PSUM"))
        self.ps_go = ctx.enter_context(tc.tile_pool(name="ps_go", bufs=1, space="PSUM"))
        self.ps_zt = ctx.enter_context(tc.tile_pool(name="ps_zt", bufs=1, space="PSUM"))
        self.ps_u = ctx.enter_context(tc.tile_pool(name="ps_u", bufs=3, space="PSUM"))

    def load_consts(self):
        nc, const = self.nc, self.const
        self.wq_sb = const.tile([128, 8, K], bf16)  # [d2%128, d2//128, key]
        self.bq_sb = const.tile([128, 4], f32)
        self.wkT_sb = const.tile([128, 4, D], bf16)  # [key%128, key//128, d]
        self.wv_sb = const.tile([128, 4, V], bf16)  # [d%128, d//128, v]
        self.bvb_sb = const.tile([128, V], f32)
        self.ones_sb = const.tile([128, 1], f32)
        nc.sync.dma_start(self.wq_sb[:], self.wq_d.rearrange("(c p) k -> p c k", p=128))
        nc.sync.dma_start(self.bq_sb[:], self.bq_d[:])
        nc.vector.memset(self.ones_sb[:], 1.0)
        nc.sync.dma_start(self.wkT_sb[:], self.wkT_d.rearrange("(c p) d -> p c d", p=128))
        nc.sync.dma_start(self.wv_sb[:], self.wv_d.rearrange("(c p) v -> p c v", p=128))
        nc.sync.dma_start(self.bvb_sb[:], self.bvb_d[:])

    def load_fcT(self, b):
        fcT_sb = self.fcT_p.tile([128, 8, Q], bf16)
        self.nc.sync.dma_start(
            fcT_sb[:], self.fcT_d[b].rearrange("(c p) q -> p c q", p=128)
        )
        return fcT_sb

    def emit_qT(self, b, fcT_sb):
        """qT[key, q'] = (s*Wq)^T @ f_c^T + s*bq -> bf16 SBUF [128, 4, Q]."""
        nc = self.nc
        qT_sb = self.qTsb_p.tile([128, 4, Q], bf16)
        for half in range(2):
            qT_ps = self.ps_qt.tile([128, 2 * Q], f32, tag="qt")
            for mi in range(2):
                m = half * 2 + mi
                for c in range(8):
                    nc.tensor.matmul(
                        qT_ps[:, mi * Q:(mi + 1) * Q],
                        self.wq_sb[:, c, m * 128:(m + 1) * 128],
                        fcT_sb[:, c, :],
                        start=(c == 0),
                        stop=(c == 7),
                    )
            for mi in range(2):
                m = half * 2 + mi
                nc.scalar.activation(
                    qT_sb[:, m, :],
                    qT_ps[:, mi * Q:(mi + 1) * Q],
                    AF.Identity,
                    bias=self.bq_sb[:, m:m + 1],
                )
        return qT_sb

    def emit_G(self, b, qT_sb):
        """G[d, q'] = Wk @ qT -> bf16 SBUF [128, 4, Q]."""
        nc = self.nc
        G_ps = self.ps_go.tile([128, 4 * Q], f32, tag="go")
        for dt_ in range(4):
            for c in range(4):
                nc.tensor.matmul(
                    G_ps[:, dt_ * Q:(dt_ + 1) * Q],
                    self.wkT_sb[:, c, dt_ * 128:(dt_ + 1) * 128],
                    qT_sb[:, c, :],
                    start=(c == 0),
                    stop=(c == 3),
                )
        G_sb = self.Gsb_p.tile([128, 4, Q], bf16)
        for h in range(2):
            nc.scalar.copy(
                G_sb[:].rearrange("p c q -> p (c q)")[:, h * 512:(h + 1) * 512],
                G_ps[:, h * 512:(h + 1) * 512],
            )
        return G_sb

    def load_tile(self, b, t):
        nc = self.nc
        fT_t = self.fT_p.tile([128, 4, NT], bf16)  # [d%128, d//128, n]
        nc.sync.dma_start(
            fT_t[:],
            self.fT_d[b, :, t * NT:(t + 1) * NT].rearrange("(c p) n -> p c n", p=128),
        )
        fn_t = self.fn_p.tile([128, 4, D], bf16)  # [n%128, n//128, d]
        nc.sync.dma_start(
            fn_t[:],
            self.fn_d[b, t * NT:(t + 1) * NT, :].rearrange("(s p) d -> p s d", p=128),
        )
        return fT_t, fn_t

    def emit_loop(self, b, G_sb):
        """Stream 32 node sub-tiles; returns (zt_ps, S_acc)."""
        nc = self.nc
        zt_ps = self.ps_zt.tile([128, 4 * Q], f32)  # zT[d, q'] accumulator
        S_acc = self.sacc_p.tile([128, Q], f32)
        nc.vector.memset(S_acc[:], 0.0)
        tiles = {0: self.load_tile(b, 0)}

        def emit_u(i):
            t, s_ = divmod(i, 4)
            fT_t, _ = tiles[t]
            u_ps = self.ps_u.tile([128, Q], f32, tag="u")
            for c in range(4):
                nc.tensor.matmul(
                    u_ps[:],
                    fT_t[:, c, s_ * 128:(s_ + 1) * 128],
                    G_sb[:, c, :],
                    start=(c == 0),
                    stop=(c == 3),
                )
            return u_ps

        pending = None  # (i, p_sb)
        u_ps = emit_u(0)
        for i in range(NSUB):
            t, s_ = divmod(i, 4)
            if s_ == 0 and t + 1 < N // NT:
                tiles[t + 1] = self.load_tile(b, t + 1)
            p_sb = self.p_p.tile([128, Q], bf16)
            nc.scalar.activation(p_sb[:], u_ps[:], AF.Exp)
            nc.vector.tensor_add(S_acc[:], S_acc[:], p_sb[:])
            if i + 1 < NSUB:
                u_ps = emit_u(i + 1)
            # zt quarters share PSUM banks in pairs (256 f32 cols = half a
            # 2KB bank): a start=True pending-zeroes the whole bank, so only
            # the first quarter in each bank starts and the last one stops.
            first = i == 0
            last = i == NSUB - 1
            fn_t = tiles[t][1]
            for dt_ in range(4):
                nc.tensor.matmul(
                    zt_ps[:, dt_ * Q:(dt_ + 1) * Q],
                    fn_t[:, s_, dt_ * 128:(dt_ + 1) * 128],
                    p_sb[:],
                    start=first and dt_ % 2 == 0,
                    stop=last and dt_ % 2 == 1,
                )
        return zt_ps, S_acc

    def emit_tail(self, b, zt_ps, S_acc):
        """out = zT^T @ Wv / S + bv, stored to DRAM."""
        nc = self.nc
        zT_sb = self.ztsb_p.tile([128, 4, Q], bf16)
        zflat = zT_sb[:].rearrange("p c q -> p (c q)")
        for h in range(2):
            nc.scalar.copy(zflat[:, h * 512:(h + 1) * 512], zt_ps[:, h * 512:(h + 1) * 512])
        # fold S_acc's 128 lanes: S[q'] = ones^T-contraction per q'-half.
        # Both columns share one PSUM bank; groups are sequential singles.
        s2_ps = self.ps_u.tile([128, 2], f32, tag="u")
        for qt in range(2):
            nc.tensor.matmul(
                s2_ps[:, qt:qt + 1],
                S_acc[:, qt * 128:(qt + 1) * 128],
                self.ones_sb[:],
                start=True,
                stop=True,
            )
        r_sb = self.small_p.tile([128, 2], f32, tag="rsb")
        nc.vector.reciprocal(r_sb[:], s2_ps[:])
        out_ps = self.ps_go.tile([128, 2 * V], f32, tag="go")
        for qt in range(2):
            for c in range(4):
                nc.tensor.matmul(
                    out_ps[:, qt * V:(qt + 1) * V],
                    zT_sb[:, c, qt * 128:(qt + 1) * 128],
                    self.wv_sb[:, c, :],
                    start=(c == 0),
                    stop=(c == 3),
                )
        for qt in range(2):
            o_sb = self.osb_p.tile([128, V], f32)
            nc.vector.tensor_scalar_mul(
                o_sb[:], out_ps[:, qt * V:(qt + 1) * V], r_sb[:, qt:qt + 1]
            )
            nc.vector.tensor_add(o_sb[:], o_sb[:], self.bvb_sb[:])
            nc.sync.dma_start(self.out_d[b, qt * 128:(qt + 1) * 128, :], o_sb[:])


def _emit(nc, tc, ctx, *tensors):
    em = _Emitter(nc, tc, ctx, tensors)
    em.load_consts()
    fcT = em.load_fcT(0)
    qT = em.emit_qT(0, fcT)
    G = em.emit_G(0, qT)
    for b in range(BPC):
        zt_ps, S_acc = em.emit_loop(b, G)
        # emit next batch's phase A before this batch's tail so PE has
        # independent work while the tail's ACT/DVE chain drains.
        if b + 1 < BPC:
            fcT = em.load_fcT(b + 1)
            qT = em.emit_qT(b + 1, fcT)
        em.emit_tail(b, zt_ps, S_acc)
        if b + 1 < BPC:
            G = em.emit_G(b + 1, qT)


_NC_CACHE = None


def build_nc():
    global _NC_CACHE
    if _NC_CACHE is not None:
        return _NC_CACHE
    nc = bacc.Bacc("TRN2", target_bir_lowering=False, debug=False)
    fcT_d = nc.declare_dram_parameter("fcT", [BPC, D2, Q], bf16, isOutput=False)
    fT_d = nc.declare_dram_parameter("fT", [BPC, D, N], bf16, isOutput=False)
    fn_d = nc.declare_dram_parameter("fn", [BPC, N, D], bf16, isOutput=False)
    wq_d = nc.declare_dram_parameter("wq", [D2, K], bf16, isOutput=False)
    wkT_d = nc.declare_dram_parameter("wkT", [K, D], bf16, isOutput=False)
    wv_d = nc.declare_dram_parameter("wv", [D, V], bf16, isOutput=False)
    bq_d = nc.declare_dram_parameter("bq", [128, 4], f32, isOutput=False)
    bvb_d = nc.declare_dram_parameter("bvb", [128, V], f32, isOutput=False)
    out_d = nc.declare_dram_parameter("out", [BPC, Q, V], f32, isOutput=True)
    with tile.TileContext(nc) as tc:
        with ExitStack() as ctx:
            _emit(nc, tc, ctx, fcT_d, fT_d, fn_d, wq_d, wkT_d, wv_d, bq_d, bvb_d, out_d)
    nc.compile()
    _NC_CACHE = nc
    return nc


def make_in_maps(f_c, f, Wq, bq, Wk, bk, Wv, bv):
    s = 1.0 / math.sqrt(K)
    f_c = np.asarray(f_c, dtype=np.float32)
    f = np.asarray(f, dtype=np.float32)
    wq_h = (np.asarray(Wq, dtype=np.float32) * s).astype(BF16)
    wkT_h = np.ascontiguousarray(np.asarray(Wk, dtype=np.float32).T).astype(BF16)
    wv_h = np.asarray(Wv, dtype=np.float32).astype(BF16)
    bq_h = np.ascontiguousarray(
        (np.asarray(bq, dtype=np.float32) * s).reshape(4, 128).T
    ).astype(np.float32)
    bvb_h = np.ascontiguousarray(
        np.broadcast_to(np.asarray(bv, dtype=np.float32), (128, V))
    )
    fn_bf = f.astype(BF16)  # [B, N, D]
    fT_bf = np.ascontiguousarray(fn_bf.transpose(0, 2, 1))  # [B, D, N]
    fcT_bf = np.ascontiguousarray(f_c.astype(BF16).transpose(0, 2, 1))  # [B, D2, Q]
    in_maps = []
    for core in range(NCORES):
        sl = slice(core * BPC, (core + 1) * BPC)
        in_maps.append(
            {
                "fcT": np.ascontiguousarray(fcT_bf[sl]),
                "fT": np.ascontiguousarray(fT_bf[sl]),
                "fn": np.ascontiguousarray(fn_bf[sl]),
                "wq": wq_h,
                "wkT": wkT_h,
                "wv": wv_h,
                "bq": bq_h,
                "bvb": bvb_h,
            }
        )
    return in_maps


def run(f_c, f, Wq, bq, Wk, bk, Wv, bv, **spmd_kwargs):
    nc = build_nc()
    in_maps = make_in_maps(f_c, f, Wq, bq, Wk, bk, Wv, bv)
    res = run_bass_kernel_spmd(nc, in_maps, list(range(NCORES)), **spmd_kwargs)
    out = np.concatenate([res.results[c]["out"] for c in range(NCORES)], axis=0)
    return out.astype(np.float32), res


def kernel(f_c, f, Wq, bq, Wk, bk, Wv, bv):
    out, _ = run(f_c, f, Wq, bq, Wk, bk, Wv, bv)
    return out


# revision 11
# speedup vs baseline: 1.3378x; 1.1643x over previous
"""Trainium2 Bass kernel for nn_Agentembedding (cross-attention agent embedding).

Reference computation (per batch b):
    q = f_c @ Wq + bq                  # [256, 512]
    k = f @ Wk + bk                    # [4096, 512]
    v = f @ Wv + bv                    # [4096, 512]
    u = (k @ q^T) / sqrt(512)          # [4096, 256]
    p = softmax(u, axis=0)             # over the 4096 nodes
    out = p^T @ v                      # [256, 512]

Optimizations used here:
  * Data parallel over batch: 32 batches -> 4 per NeuronCore across 8 cores.
  * Low-rank associativity: since Q=256 < 512,
        u = f @ G  with G = Wk @ (s*q)^T           (never materialize k)
        num = (p^T @ f) @ Wv                       (never materialize v)
    cutting matmul FLOPs ~5x vs the naive order.
  * Softmax-invariance: per-query constants cancel, so the bk.q logit term
    and the max-subtraction are dropped (logits have tiny magnitude), and
    bv is added after the normalization: out = zT^T@Wv / S + bv.
  * bf16 matmul inputs (fp32 PSUM accumulation), activations pre-cast and
    pre-transposed on host so no on-chip transposes are needed.
  * S (softmax denominators) accumulated on DVE (p-tile adds) with a final
    128-lane fold matmul, instead of per-tile N=1 matmuls on PE.
  * u(i+1) matmuls emitted before zt(i) so PE covers the exp(i) latency.
"""

import sys

sys.path.insert(0, "/opt/trn_rl_repo")

import math
from contextlib import ExitStack

import ml_dtypes
import numpy as np

import concourse.bass as bass
import concourse.tile as tile
from concourse import bacc, mybir
from concourse.bass_utils import run_bass_kernel_spmd

BF16 = ml_dtypes.bfloat16

B, Q, N, D, K, V = 32, 256, 4096, 512, 512, 512
D2 = 2 * D  # f_c feature dim (1024)
NCORES = 8
BPC = B // NCORES  # batches per core
NT = 512  # node tile (outer); 4 sub-tiles of 128 inside
NSUB = N // 128  # 32 sub-tiles per batch

f32 = mybir.dt.float32
bf16 = mybir.dt.bfloat16
AF = mybir.ActivationFunctionType


class _Emitter:
    def __init__(self, nc, tc, ctx, tensors):
        self.nc = nc
        self.tc = tc
        (self.fcT_d, self.fT_d, self.fn_d, self.wq_d, self.wkT_d, self.wv_d,
         self.bq_d, self.bvb_d, self.out_d) = tensors

        self.const = ctx.enter_context(tc.tile_pool(name="const", bufs=1))
        self.fcT_p = ctx.enter_context(tc.tile_pool(name="fcT", bufs=2))
        self.qTsb_p = ctx.enter_context(tc.tile_pool(name="qTsb", bufs=2))
        self.Gsb_p = ctx.enter_context(tc.tile_pool(name="Gsb", bufs=2))
        self.fT_p = ctx.enter_context(tc.tile_pool(name="fTp", bufs=3))
        self.fn_p = ctx.enter_context(tc.tile_pool(name="fnp", bufs=3))
        self.p_p = ctx.enter_context(tc.tile_pool(name="pp", bufs=4))
        self.sacc_p = ctx.enter_context(tc.tile_pool(name="sacc", bufs=2))
        self.ztsb_p = ctx.enter_context(tc.tile_pool(name="ztsb", bufs=2))
        self.osb_p = ctx.enter_context(tc.tile_pool(name="osb", bufs=2))
        self.small_p = ctx.enter_context(tc.tile_pool(name="small", bufs=2))
        # PSUM budget (8 banks):
        #   qT (half at a time): 1; {G, out} shared tag: 2; zt: 2; u: 3.
        self.ps_qt = ctx.enter_context(tc.tile_pool(name="ps_qt", bufs=1, space="PSUM"))
        self.ps_go = ctx.enter_context(tc.tile_pool(name="ps_go", bufs=1, space="PSUM"))
        self.ps_zt = ctx.enter_context(tc.tile_pool(name="ps_zt", bufs=1, space="PSUM"))
        self.ps_u = ctx.enter_context(tc.tile_pool(name="ps_u", bufs=3, space="PSUM"))

    def load_consts_first(self):
        """Only what phase A of batch 0 needs, so PE can start ASAP."""
        nc, const = self.nc, self.const
        self.wq_sb = const.tile([128, 8, K], bf16)  # [d2%128, d2//128, key]
        self.bq_sb = const.tile([128, 4], f32)
        self.ones_sb = const.tile([128, 1], f32)
        nc.sync.dma_start(self.wq_sb[:], self.wq_d.rearrange("(c p) k -> p c k", p=128))
        nc.sync.dma_start(self.bq_sb[:], self.bq_d[:])
        nc.vector.memset(self.ones_sb[:], 1.0)

    def load_consts_rest(self):
        nc, const = self.nc, self.const
        self.wkT_sb = const.tile([128, 4, D], bf16)  # [key%128, key//128, d]
        self.wv_sb = const.tile([128, 4, V], bf16)  # [d%128, d//128, v]
        self.bvb_sb = const.tile([128, V], f32)
        nc.sync.dma_start(self.wkT_sb[:], self.wkT_d.rearrange("(c p) d -> p c d", p=128))
        nc.sync.dma_start(self.wv_sb[:], self.wv_d.rearrange("(c p) v -> p c v", p=128))
        nc.sync.dma_start(self.bvb_sb[:], self.bvb_d[:])

    def load_fcT(self, b):
        fcT_sb = self.fcT_p.tile([128, 8, Q], bf16)
        self.nc.sync.dma_start(
            fcT_sb[:], self.fcT_d[b].rearrange("(c p) q -> p c q", p=128)
        )
        return fcT_sb

    def emit_qT(self, b, fcT_sb):
        """qT[key, q'] = (s*Wq)^T @ f_c^T + s*bq -> bf16 SBUF [128, 4, Q]."""
        nc = self.nc
        qT_sb = self.qTsb_p.tile([128, 4, Q], bf16)
        for half in range(2):
            qT_ps = self.ps_qt.tile([128, 2 * Q], f32, tag="qt")
            for mi in range(2):
                m = half * 2 + mi
                for c in range(8):
                    nc.tensor.matmul(
                        qT_ps[:, mi * Q:(mi + 1) * Q],
                        self.wq_sb[:, c, m * 128:(m + 1) * 128],
                        fcT_sb[:, c, :],
                        start=(c == 0),
                        stop=(c == 7),
                    )
            for mi in range(2):
                m = half * 2 + mi
                nc.scalar.activation(
                    qT_sb[:, m, :],
                    qT_ps[:, mi * Q:(mi + 1) * Q],
                    AF.Identity,
                    bias=self.bq_sb[:, m:m + 1],
                )
        return qT_sb

    def emit_G(self, b, qT_sb):
        """G[d, q'] = Wk @ qT -> bf16 SBUF [128, 4, Q]."""
        nc = self.nc
        G_ps = self.ps_go.tile([128, 4 * Q], f32, tag="go")
        for dt_ in range(4):
            for c in range(4):
                nc.tensor.matmul(
                    G_ps[:, dt_ * Q:(dt_ + 1) * Q],
                    self.wkT_sb[:, c, dt_ * 128:(dt_ + 1) * 128],
                    qT_sb[:, c, :],
                    start=(c == 0),
                    stop=(c == 3),
                )
        G_sb = self.Gsb_p.tile([128, 4, Q], bf16)
        for h in range(2):
            nc.scalar.copy(
                G_sb[:].rearrange("p c q -> p (c q)")[:, h * 512:(h + 1) * 512],
                G_ps[:, h * 512:(h + 1) * 512],
            )
        return G_sb

    def load_tile(self, b, t):
        nc = self.nc
        fT_t = self.fT_p.tile([128, 4, NT], bf16)  # [d%128, d//128, n]
        nc.sync.dma_start(
            fT_t[:],
            self.fT_d[b, :, t * NT:(t + 1) * NT].rearrange("(c p) n -> p c n", p=128),
        )
        fn_t = self.fn_p.tile([128, 4, D], bf16)  # [n%128, n//128, d]
        nc.sync.dma_start(
            fn_t[:],
            self.fn_d[b, t * NT:(t + 1) * NT, :].rearrange("(s p) d -> p s d", p=128),
        )
        return fT_t, fn_t

    def emit_loop(self, b, G_sb, preloaded=None):
        """Stream 32 node sub-tiles; returns (zt_ps, S_acc)."""
        nc = self.nc
        zt_ps = self.ps_zt.tile([128, 4 * Q], f32)  # zT[d, q'] accumulator
        S_acc = self.sacc_p.tile([128, Q], f32)
        nc.vector.memset(S_acc[:], 0.0)
        tiles = preloaded if preloaded else {0: self.load_tile(b, 0)}

        def emit_u(i):
            t, s_ = divmod(i, 4)
            fT_t, _ = tiles[t]
            u_ps = self.ps_u.tile([128, Q], f32, tag="u")
            for c in range(4):
                nc.tensor.matmul(
                    u_ps[:],
                    fT_t[:, c, s_ * 128:(s_ + 1) * 128],
                    G_sb[:, c, :],
                    start=(c == 0),
                    stop=(c == 3),
                )
            return u_ps

        pending = None  # (i, p_sb)
        u_ps = emit_u(0)
        for i in range(NSUB):
            t, s_ = divmod(i, 4)
            if s_ == 0 and t + 1 < N // NT and t + 1 not in tiles:
                tiles[t + 1] = self.load_tile(b, t + 1)
            p_sb = self.p_p.tile([128, Q], bf16)
            nc.scalar.activation(p_sb[:], u_ps[:], AF.Exp)
            nc.vector.tensor_add(S_acc[:], S_acc[:], p_sb[:])
            if i + 1 < NSUB:
                u_ps = emit_u(i + 1)
            # zt quarters share PSUM banks in pairs (256 f32 cols = half a
            # 2KB bank): a start=True pending-zeroes the whole bank, so only
            # the first quarter in each bank starts and the last one stops.
            first = i == 0
            last = i == NSUB - 1
            fn_t = tiles[t][1]
            for dt_ in range(4):
                nc.tensor.matmul(
                    zt_ps[:, dt_ * Q:(dt_ + 1) * Q],
                    fn_t[:, s_, dt_ * 128:(dt_ + 1) * 128],
                    p_sb[:],
                    start=first and dt_ % 2 == 0,
                    stop=last and dt_ % 2 == 1,
                )
        return zt_ps, S_acc

    def emit_tail(self, b, zt_ps, S_acc):
        """out = zT^T @ Wv / S + bv, stored to DRAM."""
        nc = self.nc
        zT_sb = self.ztsb_p.tile([128, 4, Q], bf16)
        zflat = zT_sb[:].rearrange("p c q -> p (c q)")
        for h in range(2):
            nc.scalar.copy(zflat[:, h * 512:(h + 1) * 512], zt_ps[:, h * 512:(h + 1) * 512])
        # fold S_acc's 128 lanes: S[q'] = ones^T-contraction per q'-half.
        # Both columns share one PSUM bank; groups are sequential singles.
        s2_ps = self.ps_u.tile([128, 2], f32, tag="u")
        for qt in range(2):
            nc.tensor.matmul(
                s2_ps[:, qt:qt + 1],
                S_acc[:, qt * 128:(qt + 1) * 128],
                self.ones_sb[:],
                start=True,
                stop=True,
            )
        r_sb = self.small_p.tile([128, 2], f32, tag="rsb")
        nc.vector.reciprocal(r_sb[:], s2_ps[:])
        out_ps = self.ps_go.tile([128, 2 * V], f32, tag="go")
        for qt in range(2):
            for c in range(4):
                nc.tensor.matmul(
                    out_ps[:, qt * V:(qt + 1) * V],
                    zT_sb[:, c, qt * 128:(qt + 1) * 128],
                    self.wv_sb[:, c, :],
                    start=(c == 0),
                    stop=(c == 3),
                )
        for qt in range(2):
            o_sb = self.osb_p.tile([128, V], f32)
            nc.vector.tensor_scalar_mul(
                o_sb[:], out_ps[:, qt * V:(qt + 1) * V], r_sb[:, qt:qt + 1]
            )
            nc.vector.tensor_add(o_sb[:], o_sb[:], self.bvb_sb[:])
            nc.sync.dma_start(self.out_d[b, qt * 128:(qt + 1) * 128, :], o_sb[:])


def _emit(nc, tc, ctx, *tensors):
    em = _Emitter(nc, tc, ctx, tensors)
    # DMA queue order is emission order: phase-A needs (wq, bq, fcT) first,
    # then batch 0's first node tiles, then the remaining constants.
    em.load_consts_first()
    fcT = em.load_fcT(0)
    preloaded = {0: em.load_tile(0, 0)}
    em.load_consts_rest()
    preloaded[1] = em.load_tile(0, 1)
    qT = em.emit_qT(0, fcT)
    G = em.emit_G(0, qT)
    for b in range(BPC):
        zt_ps, S_acc = em.emit_loop(b, G, preloaded if b == 0 else None)
        # emit next batch's phase A before this batch's tail so PE has
        # independent work while the tail's ACT/DVE chain drains.
        if b + 1 < BPC:
            fcT = em.load_fcT(b + 1)
            qT = em.emit_qT(b + 1, fcT)
        em.emit_tail(b, zt_ps, S_acc)
        if b + 1 < BPC:
            G = em.emit_G(b + 1, qT)


_NC_CACHE = None


def build_nc():
    global _NC_CACHE
    if _NC_CACHE is not None:
        return _NC_CACHE
    nc = bacc.Bacc("TRN2", target_bir_lowering=False, debug=False)
    fcT_d = nc.declare_dram_parameter("fcT", [BPC, D2, Q], bf16, isOutput=False)
    fT_d = nc.declare_dram_parameter("fT", [BPC, D, N], bf16, isOutput=False)
    fn_d = nc.declare_dram_parameter("fn", [BPC, N, D], bf16, isOutput=False)
    wq_d = nc.declare_dram_parameter("wq", [D2, K], bf16, isOutput=False)
    wkT_d = nc.declare_dram_parameter("wkT", [K, D], bf16, isOutput=False)
    wv_d = nc.declare_dram_parameter("wv", [D, V], bf16, isOutput=False)
    bq_d = nc.declare_dram_parameter("bq", [128, 4], f32, isOutput=False)
    bvb_d = nc.declare_dram_parameter("bvb", [128, V], f32, isOutput=False)
    out_d = nc.declare_dram_parameter("out", [BPC, Q, V], f32, isOutput=True)
    with tile.TileContext(nc) as tc:
        with ExitStack() as ctx:
            _emit(nc, tc, ctx, fcT_d, fT_d, fn_d, wq_d, wkT_d, wv_d, bq_d, bvb_d, out_d)
    nc.compile()
    _NC_CACHE = nc
    return nc


def make_in_maps(f_c, f, Wq, bq, Wk, bk, Wv, bv):
    s = 1.0 / math.sqrt(K)
    f_c = np.asarray(f_c, dtype=np.float32)
    f = np.asarray(f, dtype=np.float32)
    wq_h = (np.asarray(Wq, dtype=np.float32) * s).astype(BF16)
    wkT_h = np.ascontiguousarray(np.asarray(Wk, dtype=np.float32).T).astype(BF16)
    wv_h = np.asarray(Wv, dtype=np.float32).astype(BF16)
    bq_h = np.ascontiguousarray(
        (np.asarray(bq, dtype=np.float32) * s).reshape(4, 128).T
    ).astype(np.float32)
    bvb_h = np.ascontiguousarray(
        np.broadcast_to(np.asarray(bv, dtype=np.float32), (128, V))
    )
    fn_bf = f.astype(BF16)  # [B, N, D]
    fT_bf = np.ascontiguousarray(fn_bf.transpose(0, 2, 1))  # [B, D, N]
    fcT_bf = np.ascontiguousarray(f_c.astype(BF16).transpose(0, 2, 1))  # [B, D2, Q]
    in_maps = []
    for core in range(NCORES):
        sl = slice(core * BPC, (core + 1) * BPC)
        in_maps.append(
            {
                "fcT": np.ascontiguousarray(fcT_bf[sl]),
                "fT": np.ascontiguousarray(fT_bf[sl]),
                "fn": np.ascontiguousarray(fn_bf[sl]),
                "wq": wq_h,
                "wkT": wkT_h,
                "wv": wv_h,
                "bq": bq_h,
                "bvb": bvb_h,
            }
        )
    return in_maps


def run(f_c, f, Wq, bq, Wk, bk, Wv, bv, **spmd_kwargs):
    nc = build_nc()
    in_maps = make_in_maps(f_c, f, Wq, bq, Wk, bk, Wv, bv)
    res = run_bass_kernel_spmd(nc, in_maps, list(range(NCORES)), **spmd_kwargs)
    out = np.concatenate([res.results[c]["out"] for c in range(NCORES)], axis=0)
    return out.astype(np.float32), res


def kernel(f_c, f, Wq, bq, Wk, bk, Wv, bv):
    out, _ = run(f_c, f, Wq, bq, Wk, bk, Wv, bv)
    return out


# revision 26
# speedup vs baseline: 1.3502x; 1.0093x over previous
"""Trainium2 Bass kernel for nn_Agentembedding (cross-attention agent embedding).

Reference computation (per batch b):
    q = f_c @ Wq + bq                  # [256, 512]
    k = f @ Wk + bk                    # [4096, 512]
    v = f @ Wv + bv                    # [4096, 512]
    u = (k @ q^T) / sqrt(512)          # [4096, 256]
    p = softmax(u, axis=0)             # over the 4096 nodes
    out = p^T @ v                      # [256, 512]

Optimizations used here:
  * Data parallel over batch: 32 batches -> 4 per NeuronCore across 8 cores.
  * Low-rank associativity: since Q=256 < 512,
        u = f @ G  with G = Wk @ (s*q)^T           (never materialize k)
        num = (p^T @ f) @ Wv                       (never materialize v)
    cutting matmul FLOPs ~5x vs the naive order.
  * Softmax-invariance: per-query constants cancel, so the bk.q logit term
    and the max-subtraction are dropped (logits have tiny magnitude), and
    bv is added after the normalization: out = zT^T@Wv / S + bv.
  * bf16 matmul inputs (fp32 PSUM accumulation), activations pre-cast and
    pre-transposed on host so no on-chip transposes are needed.
  * S (softmax denominators) accumulated on DVE (p-tile adds) with a final
    128-lane fold matmul, instead of per-tile N=1 matmuls on PE.
  * u(i+1) matmuls emitted before zt(i) so PE covers the exp(i) latency.
"""

import sys

sys.path.insert(0, "/opt/trn_rl_repo")

import math
from contextlib import ExitStack

import ml_dtypes
import numpy as np

import concourse.bass as bass
import concourse.tile as tile
from concourse import bacc, mybir
from concourse.bass_utils import run_bass_kernel_spmd

BF16 = ml_dtypes.bfloat16
FP8 = ml_dtypes.float8_e4m3

B, Q, N, D, K, V = 32, 256, 4096, 512, 512, 512
D2 = 2 * D  # f_c feature dim (1024)
NCORES = 8
BPC = B // NCORES  # batches per core
NT = 512  # node tile (outer); 4 sub-tiles of 128 inside
NSUB = N // 128  # 32 sub-tiles per batch
G_SCALE = 64.0  # G values (~1e-2) are subnormal in e4m3; prescale into range

f32 = mybir.dt.float32
bf16 = mybir.dt.bfloat16
fp8 = mybir.dt.float8e4
AF = mybir.ActivationFunctionType
DR = mybir.MatmulPerfMode.DoubleRow


class _Emitter:
    def __init__(self, nc, tc, ctx, tensors):
        self.nc = nc
        self.tc = tc
        (self.fcT_d, self.fT_d, self.fn_d, self.wq_d, self.wkT_d, self.wv_d,
         self.bq_d, self.bvb_d, self.out_d) = tensors

        self.const = ctx.enter_context(tc.tile_pool(name="const", bufs=1))
        self.fcT_p = ctx.enter_context(tc.tile_pool(name="fcT", bufs=2))
        self.qTsb_p = ctx.enter_context(tc.tile_pool(name="qTsb", bufs=2))
        self.Gsb_p = ctx.enter_context(tc.tile_pool(name="Gsb", bufs=2))
        self.fT_p = ctx.enter_context(tc.tile_pool(name="fTp", bufs=3))
        self.fn_p = ctx.enter_context(tc.tile_pool(name="fnp", bufs=3))
        self.p_p = ctx.enter_context(tc.tile_pool(name="pp", bufs=4))
        self.sacc_p = ctx.enter_context(tc.tile_pool(name="sacc", bufs=2))
        self.ztsb_p = ctx.enter_context(tc.tile_pool(name="ztsb", bufs=2))
        self.osb_p = ctx.enter_context(tc.tile_pool(name="osb", bufs=2))
        self.small_p = ctx.enter_context(tc.tile_pool(name="small", bufs=2))
        # PSUM budget (8 banks):
        #   qT (half at a time): 1; {G, out} shared tag: 2; zt: 2; u: 3.
        self.ps_qt = ctx.enter_context(tc.tile_pool(name="ps_qt", bufs=1, space="PSUM"))
        self.ps_go = ctx.enter_context(tc.tile_pool(name="ps_go", bufs=1, space="PSUM"))
        self.ps_zt = ctx.enter_context(tc.tile_pool(name="ps_zt", bufs=1, space="PSUM"))
        self.ps_u = ctx.enter_context(tc.tile_pool(name="ps_u", bufs=3, space="PSUM"))

    def load_consts_first(self):
        """Only what phase A of batch 0 needs, so PE can start ASAP."""
        nc, const = self.nc, self.const
        self.wq_sb = const.tile([128, 8, K], bf16)  # [d2%128, d2//128, key]
        self.bq_sb = const.tile([128, 4], f32)
        self.ones_sb = const.tile([128, 1], f32)
        nc.sync.dma_start(self.wq_sb[:], self.wq_d.rearrange("(c p) k -> p c k", p=128))
        nc.sync.dma_start(self.bq_sb[:], self.bq_d[:])
        nc.vector.memset(self.ones_sb[:], 1.0)

    def load_consts_rest_wkT(self):
        nc, const = self.nc, self.const
        self.wkT_sb = const.tile([128, 4, D], bf16)  # [key%128, key//128, d]
        nc.sync.dma_start(self.wkT_sb[:], self.wkT_d.rearrange("(c p) d -> p c d", p=128))

    def load_consts_rest_wv(self):
        nc, const = self.nc, self.const
        self.wv_sb = const.tile([128, 4, V], bf16)  # [d%128, d//128, v]
        self.bvb_sb = const.tile([128, V], f32)
        nc.sync.dma_start(self.wv_sb[:], self.wv_d.rearrange("(c p) v -> p c v", p=128))
        nc.sync.dma_start(self.bvb_sb[:], self.bvb_d[:])

    def load_fcT(self, b):
        fcT_sb = self.fcT_p.tile([128, 8, Q], bf16)
        self.nc.sync.dma_start(
            fcT_sb[:], self.fcT_d[b].rearrange("(c p) q -> p c q", p=128)
        )
        return fcT_sb

    def emit_qT(self, b, fcT_sb):
        """qT[key, q'] = (s*Wq)^T @ f_c^T + s*bq -> bf16 SBUF [128, 4, Q]."""
        nc = self.nc
        qT_sb = self.qTsb_p.tile([128, 4, Q], bf16)
        for half in range(2):
            qT_ps = self.ps_qt.tile([128, 2 * Q], f32, tag="qt")
            for mi in range(2):
                m = half * 2 + mi
                for c in range(8):
                    nc.tensor.matmul(
                        qT_ps[:, mi * Q:(mi + 1) * Q],
                        self.wq_sb[:, c, m * 128:(m + 1) * 128],
                        fcT_sb[:, c, :],
                        start=(c == 0),
                        stop=(c == 7),
                    )
            for mi in range(2):
                m = half * 2 + mi
                nc.scalar.activation(
                    qT_sb[:, m, :],
                    qT_ps[:, mi * Q:(mi + 1) * Q],
                    AF.Identity,
                    bias=self.bq_sb[:, m:m + 1],
                )
        return qT_sb

    def emit_G(self, b, qT_sb):
        """G[d, q'] = Wk @ qT -> bf16 SBUF [128, 4, Q]."""
        nc = self.nc
        G_ps = self.ps_go.tile([128, 4 * Q], f32, tag="go")
        for dt_ in range(4):
            for c in range(4):
                nc.tensor.matmul(
                    G_ps[:, dt_ * Q:(dt_ + 1) * Q],
                    self.wkT_sb[:, c, dt_ * 128:(dt_ + 1) * 128],
                    qT_sb[:, c, :],
                    start=(c == 0),
                    stop=(c == 3),
                )
        G_sb = self.Gsb_p.tile([128, 4, Q], bf16)
        for h in range(2):
            nc.scalar.copy(
                G_sb[:].rearrange("p c q -> p (c q)")[:, h * 512:(h + 1) * 512],
                G_ps[:, h * 512:(h + 1) * 512],
            )
        return G_sb

    def load_tile(self, b, t):
        nc = self.nc
        fT_t = self.fT_p.tile([128, 4, NT], bf16)  # [d%128, d//128, n]
        nc.sync.dma_start(
            fT_t[:],
            self.fT_d[b, :, t * NT:(t + 1) * NT].rearrange("(c p) n -> p c n", p=128),
        )
        fn_t = self.fn_p.tile([128, 4, D], bf16)  # [n%128, n//128, d]
        nc.sync.dma_start(
            fn_t[:],
            self.fn_d[b, t * NT:(t + 1) * NT, :].rearrange("(s p) d -> p s d", p=128),
        )
        return fT_t, fn_t

    def emit_loop(self, b, G_sb, preloaded=None):
        """Stream 32 node sub-tiles; returns (zt_ps, S_acc)."""
        nc = self.nc
        zt_ps = self.ps_zt.tile([128, 4 * Q], f32)  # zT[d, q'] accumulator
        S_acc = self.sacc_p.tile([128, Q], f32)
        nc.vector.memset(S_acc[:], 0.0)
        tiles = preloaded if preloaded else {0: self.load_tile(b, 0)}

        def emit_u(i):
            t, s_ = divmod(i, 4)
            fT_t, _ = tiles[t]
            u_ps = self.ps_u.tile([128, Q], f32, tag="u")
            for c in range(4):
                nc.tensor.matmul(
                    u_ps[:],
                    fT_t[:, c, s_ * 128:(s_ + 1) * 128],
                    G_sb[:, c, :],
                    start=(c == 0),
                    stop=(c == 3),
                )
            return u_ps

        pending = None  # (i, p_sb)
        u_ps = emit_u(0)
        for i in range(NSUB):
            t, s_ = divmod(i, 4)
            if s_ == 0 and t + 1 < N // NT and t + 1 not in tiles:
                tiles[t + 1] = self.load_tile(b, t + 1)
            p_sb = self.p_p.tile([128, Q], bf16)
            nc.scalar.activation(p_sb[:], u_ps[:], AF.Exp)
            nc.vector.tensor_add(S_acc[:], S_acc[:], p_sb[:])
            if i + 1 < NSUB:
                u_ps = emit_u(i + 1)
            # zt quarters share PSUM banks in pairs (256 f32 cols = half a
            # 2KB bank): a start=True pending-zeroes the whole bank, so only
            # the first quarter in each bank starts and the last one stops.
            first = i == 0
            last = i == NSUB - 1
            fn_t = tiles[t][1]
            for dt_ in range(4):
                nc.tensor.matmul(
                    zt_ps[:, dt_ * Q:(dt_ + 1) * Q],
                    fn_t[:, s_, dt_ * 128:(dt_ + 1) * 128],
                    p_sb[:],
                    start=first and dt_ % 2 == 0,
                    stop=last and dt_ % 2 == 1,
                )
        return zt_ps, S_acc

    def emit_tail(self, b, zt_ps, S_acc):
        """out = zT^T @ Wv / S + bv, stored to DRAM."""
        nc = self.nc
        zT_sb = self.ztsb_p.tile([128, 4, Q], bf16)
        zflat = zT_sb[:].rearrange("p c q -> p (c q)")
        for h in range(2):
            nc.scalar.copy(zflat[:, h * 512:(h + 1) * 512], zt_ps[:, h * 512:(h + 1) * 512])
        # fold S_acc's 128 lanes: S[q'] = ones^T-contraction per q'-half.
        # Both columns share one PSUM bank; groups are sequential singles.
        s2_ps = self.ps_u.tile([128, 2], f32, tag="u")
        for qt in range(2):
            nc.tensor.matmul(
                s2_ps[:, qt:qt + 1],
                S_acc[:, qt * 128:(qt + 1) * 128],
                self.ones_sb[:],
                start=True,
                stop=True,
            )
        r_sb = self.small_p.tile([128, 2], f32, tag="rsb")
        nc.vector.reciprocal(r_sb[:], s2_ps[:])
        out_ps = self.ps_go.tile([128, 2 * V], f32, tag="go")
        for qt in range(2):
            for c in range(4):
                nc.tensor.matmul(
                    out_ps[:, qt * V:(qt + 1) * V],
                    zT_sb[:, c, qt * 128:(qt + 1) * 128],
                    self.wv_sb[:, c, :],
                    start=(c == 0),
                    stop=(c == 3),
                )
        for qt in range(2):
            o_sb = self.osb_p.tile([128, V], f32)
            nc.vector.tensor_scalar_mul(
                o_sb[:], out_ps[:, qt * V:(qt + 1) * V], r_sb[:, qt:qt + 1]
            )
            nc.vector.tensor_add(o_sb[:], o_sb[:], self.bvb_sb[:])
            nc.sync.dma_start(self.out_d[b, qt * 128:(qt + 1) * 128, :], o_sb[:])


def _emit(nc, tc, ctx, *tensors):
    em = _Emitter(nc, tc, ctx, tensors)
    # DMA queue order is emission order: phase-A needs (wq, bq, fcT) first,
    # then batch 0's first node tiles, then the remaining constants.
    em.load_consts_first()
    fcT = em.load_fcT(0)
    preloaded = {0: em.load_tile(0, 0)}
    em.load_consts_rest_wkT()
    preloaded[1] = em.load_tile(0, 1)
    preloaded[2] = em.load_tile(0, 2)
    em.load_consts_rest_wv()
    qT = em.emit_qT(0, fcT)
    G = em.emit_G(0, qT)
    for b in range(BPC):
        zt_ps, S_acc = em.emit_loop(b, G, preloaded if b == 0 else None)
        # emit next batch's phase A before this batch's tail so PE has
        # independent work while the tail's ACT/DVE chain drains.
        if b + 1 < BPC:
            fcT = em.load_fcT(b + 1)
            qT = em.emit_qT(b + 1, fcT)
        em.emit_tail(b, zt_ps, S_acc)
        if b + 1 < BPC:
            G = em.emit_G(b + 1, qT)


_NC_CACHE = None


def build_nc():
    global _NC_CACHE
    if _NC_CACHE is not None:
        return _NC_CACHE
    nc = bacc.Bacc("TRN2", target_bir_lowering=False, debug=False)
    fcT_d = nc.declare_dram_parameter("fcT", [BPC, D2, Q], bf16, isOutput=False)
    fT_d = nc.declare_dram_parameter("fT", [BPC, D, N], bf16, isOutput=False)
    fn_d = nc.declare_dram_parameter("fn", [BPC, N, D], bf16, isOutput=False)
    wq_d = nc.declare_dram_parameter("wq", [D2, K], bf16, isOutput=False)
    wkT_d = nc.declare_dram_parameter("wkT", [K, D], bf16, isOutput=False)
    wv_d = nc.declare_dram_parameter("wv", [D, V], bf16, isOutput=False)
    bq_d = nc.declare_dram_parameter("bq", [128, 4], f32, isOutput=False)
    bvb_d = nc.declare_dram_parameter("bvb", [128, V], f32, isOutput=False)
    out_d = nc.declare_dram_parameter("out", [BPC, Q, V], f32, isOutput=True)
    with tile.TileContext(nc) as tc:
        with ExitStack() as ctx:
            _emit(nc, tc, ctx, fcT_d, fT_d, fn_d, wq_d, wkT_d, wv_d, bq_d, bvb_d, out_d)
    nc.compile()
    _NC_CACHE = nc
    return nc


def make_in_maps(f_c, f, Wq, bq, Wk, bk, Wv, bv):
    s = 1.0 / math.sqrt(K)
    f_c = np.asarray(f_c, dtype=np.float32)
    f = np.asarray(f, dtype=np.float32)
    wq_h = (np.asarray(Wq, dtype=np.float32) * s).astype(BF16)
    wkT_h = np.ascontiguousarray(np.asarray(Wk, dtype=np.float32).T).astype(BF16)
    wv_h = np.asarray(Wv, dtype=np.float32).astype(BF16)
    bq_h = np.ascontiguousarray(
        (np.asarray(bq, dtype=np.float32) * s).reshape(4, 128).T
    ).astype(np.float32)
    bvb_h = np.ascontiguousarray(
        np.broadcast_to(np.asarray(bv, dtype=np.float32), (128, V))
    )
    fn_bf = f.astype(BF16)  # [B, N, D]
    fT_bf = np.ascontiguousarray(fn_bf.transpose(0, 2, 1))  # [B, D, N]
    fcT_bf = np.ascontiguousarray(f_c.astype(BF16).transpose(0, 2, 1))  # [B, D2, Q]
    in_maps = []
    for core in range(NCORES):
        sl = slice(core * BPC, (core + 1) * BPC)
        in_maps.append(
            {
                "fcT": np.ascontiguousarray(fcT_bf[sl]),
                "fT": np.ascontiguousarray(fT_bf[sl]),
                "fn": np.ascontiguousarray(fn_bf[sl]),
                "wq": wq_h,
                "wkT": wkT_h,
                "wv": wv_h,
                "bq": bq_h,
                "bvb": bvb_h,
            }
        )
    return in_maps


def run(f_c, f, Wq, bq, Wk, bk, Wv, bv, **spmd_kwargs):
    nc = build_nc()
    in_maps = make_in_maps(f_c, f, Wq, bq, Wk, bk, Wv, bv)
    res = run_bass_kernel_spmd(nc, in_maps, list(range(NCORES)), **spmd_kwargs)
    out = np.concatenate([res.results[c]["out"] for c in range(NCORES)], axis=0)
    return out.astype(np.float32), res


def kernel(f_c, f, Wq, bq, Wk, bk, Wv, bv):
    out, _ = run(f_c, f, Wq, bq, Wk, bk, Wv, bv)
    return out
